# revision 19
# baseline (speedup 1.0000x reference)
"""AdaptiveAttentionLSTMCell fused kernel for one TRN2 chip (8 NeuronCores).

Math note: the reference applies softmax over a size-1 axis (zt is [B, K+1, 1],
softmax(axis=-1)), which is identically 1.0 for finite inputs. Hence
ct = sum_k v_expand[:, k, :] = v_seq.sum(axis=1) + st exactly, and the
W_z / U_z / W_h attention projections never affect the output. The kernel
therefore computes:

    z  = h_tm @ W_gates + inputs @ U_gates + b_gates          [B, 5U]
    ft,it,ot,gt = sigmoid(f,i,o,g);  at = tanh(a)
    mt = m_tm * ft + it * at
    tm = tanh(mt); ht = ot * tm; st = gt * tm
    out = (ht + st + v_seq.sum(1), ht, mt)     # (ot+gt)*tm == ht+st

Distribution: 2-way data-parallel over batch x 4-way parallel over the unit
dim (each core owns all 5 gate blocks for its 256 units, so the gate
elementwise math stays local and no collective is needed). Host reassembles
the 8 output shards (device outputs are bf16; host upcasts to f32).

Per-core schedule: the [1024, 3072] @ [3072, 1280] gate matmul runs as 6
phases (2 batch groups x 3 psum-bank-aligned column chunks); gate columns
are host-reordered to [f, i | a, o | g]. All input DMA flows through the
sync engine's HWDGE FIFO (weights/activations first, then v_seq), output
DMA through the scalar engine's ring. v_seq is reduced by in-place bf16
add trees on the vector engine; gate epilogues run group-wide on
[128, 4, 256] tiles. Matmuls run in bf16 with f32 PSUM accumulation.
"""

import numpy as np
import ml_dtypes

# Problem shape (hardcoded per the harness contract).
B, D_IN, UNITS, KF = 2048, 2048, 1024, 49
N_CORES = 8
PB, PU = 2, 4                 # batch shards x unit shards
B_L = B // PB                 # 1024 batch rows per core
U_L = UNITS // PU             # 256 units per core
K = UNITS + D_IN              # 3072 contraction dim
N_L = 5 * U_L                 # 1280 gate columns per core
P = 128                       # partitions
NB_T = B_L // P               # 8 batch tiles
NK_T = K // P                 # 24 k tiles
GRP = 4                       # batch tiles per phase group
BF16 = ml_dtypes.bfloat16

_NC_CACHE = {}


def _build_nc(with_bias):
    import concourse.bacc as bacc
    import concourse.mybir as mybir
    import concourse.tile as tile

    dt = mybir.dt
    f32, bf = dt.float32, dt.bfloat16
    Sig = mybir.ActivationFunctionType.Sigmoid
    Tanh = mybir.ActivationFunctionType.Tanh
    Add = mybir.AluOpType.add
    nc = bacc.Bacc("TRN2", target_bir_lowering=False, debug=False)

    aT = nc.dram_tensor("aT", [K, B_L], bf, kind="ExternalInput").ap()
    W = nc.dram_tensor("W", [K, N_L], bf, kind="ExternalInput").ap()
    m = nc.dram_tensor("m", [B_L, U_L], bf, kind="ExternalInput").ap()
    v = nc.dram_tensor("v", [B_L, KF, U_L], bf, kind="ExternalInput").ap()
    if with_bias:
        bb = nc.dram_tensor("bb", [P, N_L], f32, kind="ExternalInput").ap()
    o0 = nc.dram_tensor("o0", [B_L, U_L], bf, kind="ExternalOutput").ap()
    o1 = nc.dram_tensor("o1", [B_L, U_L], bf, kind="ExternalOutput").ap()
    o2 = nc.dram_tensor("o2", [B_L, U_L], bf, kind="ExternalOutput").ap()

    # column chunks within the reordered [f i a o g] gate layout
    PH = ((0, 512), (512, 512), (1024, 256))
    GROUPS = (tuple(range(0, GRP)), tuple(range(GRP, NB_T)))

    def grp_dram(ap_, g):
        # [512, 256] dram rows of group g viewed as [128, 4, 256]
        return ap_[g * GRP * P:(g + 1) * GRP * P, :].rearrange(
            "(i p) u -> p i u", p=P)

    with tile.TileContext(nc) as tc:
        with (
            tc.tile_pool(name="resident", bufs=1) as rp,
            tc.tile_pool(name="vload", bufs=2) as vp,
            tc.tile_pool(name="grp", bufs=2) as gp,
            tc.tile_pool(name="psum", bufs=8, space="PSUM") as pp,
        ):
            aT_sb = rp.tile([P, NK_T * B_L], bf)
            W_sb = rp.tile([P, NK_T * N_L], bf)
            for k in range(NK_T):
                nc.sync.dma_start(W_sb[:, k * N_L:(k + 1) * N_L], W[k * P:(k + 1) * P, :])
                nc.sync.dma_start(aT_sb[:, k * B_L:(k + 1) * B_L], aT[k * P:(k + 1) * P, :])
            if with_bias:
                bb_sb = rp.tile([P, N_L], f32)
                nc.sync.dma_start(bb_sb[:], bb[:])

            m_gs = {}
            for g in range(2):
                m_g = gp.tile([P, GRP, U_L], bf, tag="m", name=f"m_g{g}")
                nc.sync.dma_start(m_g[:], grp_dram(m, g))
                m_gs[g] = m_g

            vts = {}
            for bt in range(NB_T):
                bs = slice(bt * P, (bt + 1) * P)
                vt = vp.tile([P, KF, U_L], bf, tag="vt", name=f"vt{bt}")
                nc.sync.dma_start(vt[:, 0:25, :], v[bs, 0:25, :])
                nc.sync.dma_start(vt[:, 25:49, :], v[bs, 25:49, :])
                vts[bt] = vt

            def mm_phase(group, ph):
                n0, nw = PH[ph]
                zs = {}
                for bt in group:
                    zs[bt] = pp.tile([P, 512], f32, tag="z", name=f"z_{ph}_{bt}")
                for k in range(NK_T):
                    for bt in group:
                        lhsT = aT_sb[:, k * B_L + bt * P: k * B_L + (bt + 1) * P]
                        nc.tensor.matmul(
                            zs[bt][:, :nw],
                            lhsT,
                            W_sb[:, k * N_L + n0: k * N_L + n0 + nw],
                            start=(k == 0),
                            stop=(k == NK_T - 1),
                        )
                if with_bias:
                    for bt in group:
                        nc.vector.tensor_add(zs[bt][:, :nw], zs[bt][:, :nw],
                                             bb_sb[:, n0:n0 + nw])
                return zs


            # ---- per group: phases (PE + ACT gate reads), then finalize ----
            for g, group in enumerate(GROUPS):
                ft = gp.tile([P, GRP, U_L], bf, tag="ft", name=f"ft{g}")
                it = gp.tile([P, GRP, U_L], bf, tag="it", name=f"it{g}")
                at = gp.tile([P, GRP, U_L], bf, tag="at", name=f"at{g}")
                ot = gp.tile([P, GRP, U_L], bf, tag="ot", name=f"ot{g}")
                gt = gp.tile([P, GRP, U_L], bf, tag="gt", name=f"gt{g}")
                z0 = mm_phase(group, 0)            # f, i
                for i, bt in enumerate(group):
                    nc.scalar.activation(ft[:, i, :], z0[bt][:, 0:U_L], Sig)
                    nc.scalar.activation(it[:, i, :], z0[bt][:, U_L:2 * U_L], Sig)
                z1 = mm_phase(group, 1)            # a, o
                for i, bt in enumerate(group):
                    nc.scalar.activation(at[:, i, :], z1[bt][:, 0:U_L], Tanh)
                    nc.scalar.activation(ot[:, i, :], z1[bt][:, U_L:2 * U_L], Sig)
                z2 = mm_phase(group, 2)            # g
                for i, bt in enumerate(group):
                    nc.scalar.activation(gt[:, i, :], z2[bt][:, 0:U_L], Sig)

                # finalize: v trees at data pace with chain ops filling
                # DVE slack; chain results reuse dead tiles
                vs = gp.tile([P, GRP, U_L], bf, tag="vs", name=f"vs{g}")
                m_g = m_gs[g]
                tm = gp.tile([P, GRP, U_L], bf, tag="tm", name=f"tm{g}")

                def tree_bt(i, bt):
                    # 7-op in-place bf16 fold of 49 slices; result in vs[:, i]
                    t = vts[bt]
                    A = nc.vector.tensor_add
                    A(t[:, 1:25, :], t[:, 1:25, :], t[:, 25:49, :])
                    A(t[:, 0:12, :], t[:, 0:12, :], t[:, 13:25, :])
                    A(t[:, 0:6, :], t[:, 0:6, :], t[:, 6:12, :])
                    A(t[:, 0:3, :], t[:, 0:3, :], t[:, 3:6, :])
                    A(t[:, 0, :], t[:, 0, :], t[:, 1, :])
                    A(t[:, 0, :], t[:, 0, :], t[:, 2, :])
                    A(vs[:, i, :], t[:, 0, :], t[:, 12, :])

                tree_bt(0, group[0])
                nc.vector.tensor_mul(m_g[:], m_g[:], ft[:])      # m*ft
                nc.vector.tensor_mul(it[:], it[:], at[:])        # it*at
                tree_bt(1, group[1])
                nc.vector.tensor_add(m_g[:], m_g[:], it[:])      # mt (in m_g)
                nc.scalar.dma_start(grp_dram(o2, g), m_g[:])
                nc.scalar.activation(tm[:], m_g[:], Tanh)
                tree_bt(2, group[2])
                nc.vector.tensor_mul(it[:], ot[:], tm[:])        # ht (in it)
                nc.scalar.dma_start(grp_dram(o1, g), it[:])
                nc.vector.tensor_add(ot[:], ot[:], gt[:])        # ot+gt
                nc.vector.tensor_mul(ft[:], ot[:], tm[:])        # ht+st (in ft)
                tree_bt(3, group[3])
                nc.vector.tensor_add(ft[:], ft[:], vs[:])        # + vsum
                nc.scalar.dma_start(grp_dram(o0, g), ft[:])

    nc.compile()
    return nc


def _get_nc(with_bias):
    key = bool(with_bias)
    if key not in _NC_CACHE:
        _NC_CACHE[key] = _build_nc(key)
    return _NC_CACHE[key]


def _prepare_in_maps(inputs):
    x = np.asarray(inputs["inputs"], np.float32)
    h = np.asarray(inputs["h_tm"], np.float32)
    m = np.asarray(inputs["m_tm"], np.float32)
    v = np.asarray(inputs["v_seq"], np.float32)
    Wg = np.asarray(inputs["W_gates"], np.float32)
    Ug = np.asarray(inputs["U_gates"], np.float32)
    bg = np.asarray(inputs["b_gates"], np.float32)

    with_bias = bool(np.any(bg))
    A_T = np.ascontiguousarray(np.concatenate([h, x], axis=1).T.astype(BF16))  # [K, B]
    W_full = np.concatenate([Wg, Ug], axis=0)                                   # [K, 5U]

    in_maps = []
    for c in range(N_CORES):
        pb, pu = divmod(c, PU)
        bsl = slice(pb * B_L, (pb + 1) * B_L)
        u = np.arange(pu * U_L, (pu + 1) * U_L)
        # gate-block order [f, i, a, o, g] (reference stacks [f, i, o, g, a])
        cols = np.concatenate([j * UNITS + u for j in (0, 1, 4, 2, 3)])
        im = {
            "aT": np.ascontiguousarray(A_T[:, bsl]),
            "W": np.ascontiguousarray(W_full[:, cols].astype(BF16)),
            "m": np.ascontiguousarray(m[bsl, pu * U_L:(pu + 1) * U_L].astype(BF16)),
            "v": np.ascontiguousarray(v[bsl, :, pu * U_L:(pu + 1) * U_L].astype(BF16)),
        }
        if with_bias:
            im["bb"] = np.ascontiguousarray(
                np.broadcast_to(bg[cols], (P, N_L)).astype(np.float32))
        in_maps.append(im)
    return in_maps, with_bias


def _assemble(results):
    outs = []
    for name in ("o0", "o1", "o2"):
        full = np.empty((B, UNITS), np.float32)
        for c in range(N_CORES):
            pb, pu = divmod(c, PU)
            full[pb * B_L:(pb + 1) * B_L, pu * U_L:(pu + 1) * U_L] = \
                np.asarray(results[c][name]).astype(np.float32)
        outs.append(full)
    return tuple(outs)


def _run(inputs, **spmd_kwargs):
    from concourse.bass_utils import run_bass_kernel_spmd

    in_maps, with_bias = _prepare_in_maps(inputs)
    nc = _get_nc(with_bias)
    res = run_bass_kernel_spmd(nc, in_maps, core_ids=list(range(N_CORES)),
                               **spmd_kwargs)
    return _assemble(res.results), res


def kernel(**inputs):
    outs, _ = _run(inputs)
    return outs


# revision 20
# speedup vs baseline: 1.2242x; 1.2242x over previous
"""AdaptiveAttentionLSTMCell fused kernel for one TRN2 chip (8 NeuronCores).

Math note: the reference applies softmax over a size-1 axis (zt is [B, K+1, 1],
softmax(axis=-1)), which is identically 1.0 for finite inputs. Hence
ct = sum_k v_expand[:, k, :] = v_seq.sum(axis=1) + st exactly, and the
W_z / U_z / W_h attention projections never affect the output. The kernel
therefore computes:

    z  = h_tm @ W_gates + inputs @ U_gates + b_gates          [B, 5U]
    ft,it,ot,gt = sigmoid(f,i,o,g);  at = tanh(a)
    mt = m_tm * ft + it * at
    tm = tanh(mt); ht = ot * tm; st = gt * tm
    out = (ht + st + v_seq.sum(1), ht, mt)     # (ot+gt)*tm == ht+st

Distribution: 2-way data-parallel over batch x 4-way parallel over the unit
dim (each core owns all 5 gate blocks for its 256 units, so the gate
elementwise math stays local and no collective is needed). Host reassembles
the 8 output shards (device outputs are bf16; host upcasts to f32).

Per-core schedule: the [1024, 3072] @ [3072, 1280] gate matmul runs as 6
phases (2 batch groups x 3 psum-bank-aligned column chunks); gate columns
are host-reordered to [f, i | a, o | g]. All input DMA flows through the
sync engine's HWDGE FIFO (weights/activations first, then v_seq), output
DMA through the scalar engine's ring. v_seq is reduced by in-place bf16
add trees on the vector engine; gate epilogues run group-wide on
[128, 4, 256] tiles. Matmuls run in bf16 with f32 PSUM accumulation.
"""

import numpy as np
import ml_dtypes

# Problem shape (hardcoded per the harness contract).
B, D_IN, UNITS, KF = 2048, 2048, 1024, 49
N_CORES = 8
PB, PU = 2, 4                 # batch shards x unit shards
B_L = B // PB                 # 1024 batch rows per core
U_L = UNITS // PU             # 256 units per core
K = UNITS + D_IN              # 3072 contraction dim
N_L = 5 * U_L                 # 1280 gate columns per core
P = 128                       # partitions
NB_T = B_L // P               # 8 batch tiles
NK_T = K // P                 # 24 k tiles
GRP = 4                       # batch tiles per phase group
BF16 = ml_dtypes.bfloat16

_NC_CACHE = {}


def _build_nc(with_bias):
    import concourse.bacc as bacc
    import concourse.mybir as mybir
    import concourse.tile as tile

    dt = mybir.dt
    f32, bf = dt.float32, dt.bfloat16
    Sig = mybir.ActivationFunctionType.Sigmoid
    Tanh = mybir.ActivationFunctionType.Tanh
    Add = mybir.AluOpType.add
    nc = bacc.Bacc("TRN2", target_bir_lowering=False, debug=False)

    aT = nc.dram_tensor("aT", [K, B_L], bf, kind="ExternalInput").ap()
    W = nc.dram_tensor("W", [K, N_L], bf, kind="ExternalInput").ap()
    m = nc.dram_tensor("m", [B_L, U_L], bf, kind="ExternalInput").ap()
    v = nc.dram_tensor("v", [B_L, KF, U_L], bf, kind="ExternalInput").ap()
    if with_bias:
        bb = nc.dram_tensor("bb", [P, N_L], f32, kind="ExternalInput").ap()
    o0 = nc.dram_tensor("o0", [B_L, U_L], bf, kind="ExternalOutput").ap()
    o1 = nc.dram_tensor("o1", [B_L, U_L], bf, kind="ExternalOutput").ap()
    o2 = nc.dram_tensor("o2", [B_L, U_L], bf, kind="ExternalOutput").ap()

    # column chunks within the reordered [f i a o g] gate layout
    PH = ((0, 512), (512, 512), (1024, 256))
    GROUPS = (tuple(range(0, GRP)), tuple(range(GRP, NB_T)))

    def grp_dram(ap_, g):
        # [512, 256] dram rows of group g viewed as [128, 4, 256]
        return ap_[g * GRP * P:(g + 1) * GRP * P, :].rearrange(
            "(i p) u -> p i u", p=P)

    with tile.TileContext(nc) as tc:
        with (
            tc.tile_pool(name="resident", bufs=1) as rp,
            tc.tile_pool(name="vload", bufs=2) as vp,
            tc.tile_pool(name="vloadb", bufs=3) as vpb,
            tc.tile_pool(name="grp", bufs=2) as gp,
            tc.tile_pool(name="psum", bufs=8, space="PSUM") as pp,
        ):
            aT_sb = rp.tile([P, NK_T * B_L], bf)
            W_sb = rp.tile([P, NK_T * N_L], bf)
            # front-load one v half ahead of the residents: balances the
            # PE-paced head against the v-paced tail
            va0 = vp.tile([P, 25, U_L], bf, tag="va", name="va0")
            nc.sync.dma_start(va0[:], v[0:P, 0:25, :])
            for k in range(NK_T):
                nc.sync.dma_start(W_sb[:, k * N_L:(k + 1) * N_L], W[k * P:(k + 1) * P, :])
                nc.sync.dma_start(aT_sb[:, k * B_L:(k + 1) * B_L], aT[k * P:(k + 1) * P, :])
            if with_bias:
                bb_sb = rp.tile([P, N_L], f32)
                nc.sync.dma_start(bb_sb[:], bb[:])

            m_gs = {}
            for g in range(2):
                m_g = gp.tile([P, GRP, U_L], bf, tag="m", name=f"m_g{g}")
                nc.sync.dma_start(m_g[:], grp_dram(m, g))
                m_gs[g] = m_g

            vts = {}
            for bt in range(NB_T):
                bs = slice(bt * P, (bt + 1) * P)
                if bt == 0:
                    va = va0
                else:
                    va = vp.tile([P, 25, U_L], bf, tag="va", name=f"va{bt}")
                    nc.sync.dma_start(va[:], v[bs, 0:25, :])
                vb = vpb.tile([P, 24, U_L], bf, tag="vb", name=f"vb{bt}")
                nc.sync.dma_start(vb[:], v[bs, 25:49, :])
                vts[bt] = (va, vb)

            def mm_phase(group, ph):
                n0, nw = PH[ph]
                zs = {}
                for bt in group:
                    zs[bt] = pp.tile([P, 512], f32, tag="z", name=f"z_{ph}_{bt}")
                for k in range(NK_T):
                    for bt in group:
                        lhsT = aT_sb[:, k * B_L + bt * P: k * B_L + (bt + 1) * P]
                        nc.tensor.matmul(
                            zs[bt][:, :nw],
                            lhsT,
                            W_sb[:, k * N_L + n0: k * N_L + n0 + nw],
                            start=(k == 0),
                            stop=(k == NK_T - 1),
                        )
                if with_bias:
                    for bt in group:
                        nc.vector.tensor_add(zs[bt][:, :nw], zs[bt][:, :nw],
                                             bb_sb[:, n0:n0 + nw])
                return zs


            # ---- per group: phases (PE + ACT gate reads), then finalize ----
            for g, group in enumerate(GROUPS):
                ft = gp.tile([P, GRP, U_L], bf, tag="ft", name=f"ft{g}")
                it = gp.tile([P, GRP, U_L], bf, tag="it", name=f"it{g}")
                at = gp.tile([P, GRP, U_L], bf, tag="at", name=f"at{g}")
                ot = gp.tile([P, GRP, U_L], bf, tag="ot", name=f"ot{g}")
                gt = gp.tile([P, GRP, U_L], bf, tag="gt", name=f"gt{g}")
                z0 = mm_phase(group, 0)            # f, i
                for i, bt in enumerate(group):
                    nc.scalar.activation(ft[:, i, :], z0[bt][:, 0:U_L], Sig)
                    nc.scalar.activation(it[:, i, :], z0[bt][:, U_L:2 * U_L], Sig)
                z1 = mm_phase(group, 1)            # a, o
                for i, bt in enumerate(group):
                    nc.scalar.activation(at[:, i, :], z1[bt][:, 0:U_L], Tanh)
                    nc.scalar.activation(ot[:, i, :], z1[bt][:, U_L:2 * U_L], Sig)
                z2 = mm_phase(group, 2)            # g
                for i, bt in enumerate(group):
                    nc.scalar.activation(gt[:, i, :], z2[bt][:, 0:U_L], Sig)

                # finalize: v trees at data pace; chain ops fill DVE slack
                # and stay decoupled from v arrival; results reuse dead tiles
                vs = gp.tile([P, GRP, U_L], bf, tag="vs", name=f"vs{g}")
                m_g = m_gs[g]
                tm = gp.tile([P, GRP, U_L], bf, tag="tm", name=f"tm{g}")
                A = nc.vector.tensor_add
                M = nc.vector.tensor_mul

                def treeA(i, bt):
                    t = vts[bt][0]   # 25 slices -> partials at [0] and [12]
                    A(t[:, 0:12, :], t[:, 0:12, :], t[:, 13:25, :])
                    A(t[:, 0:6, :], t[:, 0:6, :], t[:, 6:12, :])
                    A(t[:, 0:3, :], t[:, 0:3, :], t[:, 3:6, :])
                    A(t[:, 0, :], t[:, 0, :], t[:, 1, :])
                    A(t[:, 0, :], t[:, 0, :], t[:, 2, :])
                    A(t[:, 0, :], t[:, 0, :], t[:, 12, :])

                def treeB(i, bt):
                    t = vts[bt][1]   # 24 slices -> partial at [0]
                    A(t[:, 0:12, :], t[:, 0:12, :], t[:, 12:24, :])
                    A(t[:, 0:6, :], t[:, 0:6, :], t[:, 6:12, :])
                    A(t[:, 0:3, :], t[:, 0:3, :], t[:, 3:6, :])
                    A(t[:, 0, :], t[:, 0, :], t[:, 1, :])
                    A(t[:, 0, :], t[:, 0, :], t[:, 2, :])
                    A(vs[:, i, :], vts[bt][0][:, 0, :], t[:, 0, :])

                def chains1():
                    M(m_g[:], m_g[:], ft[:])                     # m*ft
                    M(it[:], it[:], at[:])                       # it*at
                    A(m_g[:], m_g[:], it[:])                     # mt (in m_g)
                    nc.scalar.dma_start(grp_dram(o2, g), m_g[:])
                    nc.scalar.activation(tm[:], m_g[:], Tanh)

                def chains2():
                    M(it[:], ot[:], tm[:])                       # ht (in it)
                    nc.scalar.dma_start(grp_dram(o1, g), it[:])
                    A(ot[:], ot[:], gt[:])                       # ot+gt
                    M(ft[:], ot[:], tm[:])                       # ht+st (in ft)

                def final():
                    A(ft[:], ft[:], vs[:])                       # + vsum
                    nc.scalar.dma_start(grp_dram(o0, g), ft[:])

                if g == 0:
                    treeA(0, group[0])
                    chains1()
                    chains2()
                    treeB(0, group[0])
                    treeA(1, group[1]); treeB(1, group[1])
                    treeA(2, group[2]); treeB(2, group[2])
                    treeA(3, group[3]); treeB(3, group[3])
                    final()
                else:
                    treeA(0, group[0]); treeB(0, group[0])
                    treeA(1, group[1]); treeB(1, group[1])
                    chains1()
                    treeA(2, group[2]); treeB(2, group[2])
                    chains2()
                    treeA(3, group[3]); treeB(3, group[3])
                    final()

    nc.compile()
    return nc


def _get_nc(with_bias):
    key = bool(with_bias)
    if key not in _NC_CACHE:
        _NC_CACHE[key] = _build_nc(key)
    return _NC_CACHE[key]


def _prepare_in_maps(inputs):
    x = np.asarray(inputs["inputs"], np.float32)
    h = np.asarray(inputs["h_tm"], np.float32)
    m = np.asarray(inputs["m_tm"], np.float32)
    v = np.asarray(inputs["v_seq"], np.float32)
    Wg = np.asarray(inputs["W_gates"], np.float32)
    Ug = np.asarray(inputs["U_gates"], np.float32)
    bg = np.asarray(inputs["b_gates"], np.float32)

    with_bias = bool(np.any(bg))
    A_T = np.ascontiguousarray(np.concatenate([h, x], axis=1).T.astype(BF16))  # [K, B]
    W_full = np.concatenate([Wg, Ug], axis=0)                                   # [K, 5U]

    in_maps = []
    for c in range(N_CORES):
        pb, pu = divmod(c, PU)
        bsl = slice(pb * B_L, (pb + 1) * B_L)
        u = np.arange(pu * U_L, (pu + 1) * U_L)
        # gate-block order [f, i, a, o, g] (reference stacks [f, i, o, g, a])
        cols = np.concatenate([j * UNITS + u for j in (0, 1, 4, 2, 3)])
        im = {
            "aT": np.ascontiguousarray(A_T[:, bsl]),
            "W": np.ascontiguousarray(W_full[:, cols].astype(BF16)),
            "m": np.ascontiguousarray(m[bsl, pu * U_L:(pu + 1) * U_L].astype(BF16)),
            "v": np.ascontiguousarray(v[bsl, :, pu * U_L:(pu + 1) * U_L].astype(BF16)),
        }
        if with_bias:
            im["bb"] = np.ascontiguousarray(
                np.broadcast_to(bg[cols], (P, N_L)).astype(np.float32))
        in_maps.append(im)
    return in_maps, with_bias


def _assemble(results):
    outs = []
    for name in ("o0", "o1", "o2"):
        full = np.empty((B, UNITS), np.float32)
        for c in range(N_CORES):
            pb, pu = divmod(c, PU)
            full[pb * B_L:(pb + 1) * B_L, pu * U_L:(pu + 1) * U_L] = \
                np.asarray(results[c][name]).astype(np.float32)
        outs.append(full)
    return tuple(outs)


def _run(inputs, **spmd_kwargs):
    from concourse.bass_utils import run_bass_kernel_spmd

    in_maps, with_bias = _prepare_in_maps(inputs)
    nc = _get_nc(with_bias)
    res = run_bass_kernel_spmd(nc, in_maps, core_ids=list(range(N_CORES)),
                               **spmd_kwargs)
    return _assemble(res.results), res


def kernel(**inputs):
    outs, _ = _run(inputs)
    return outs
